# revision 1
# baseline (speedup 1.0000x reference)
"""Trainium2 Bass kernel for masked similar-user attention.

Computation (per batch b, position s):
    scores[u] = dot(user[b], sim[b,s,u,:])        (u = 50 similar users, d = 32)
    scores    = where(mask, -1e9, scores)
    attn      = softmax(scores)
    out[s]    = sum_u attn[u] * sim[b,s,u,:] + item[b,s]

Sharding: pure data parallel over batch (B=512 -> 64 per core, 8 cores).

Implementation: raw Bass (explicit engine streams + semaphores).  Rows =
(b, s) pairs on SBUF partitions.  All per-row operands are packed host-side
into ONE row-major DRAM tensor [sim(1600) | user(32) | maskf(50) | item(32)]
so each tile is a single contiguous-load DMA at full HBM bandwidth.  Both
contractions (over d and u) are per-partition free-dim ops on DVE; exp runs
on ACT with fused -max bias and fused denominator accumulation; stores go
out on the ACT HWDGE queue.  Every cross-engine dependency is a standalone
single-wait instruction on a monotonic semaphore (this walrus build allows
only one sync-wait per instruction).

Pipeline (per outer tile T of 128 x G rows; sems LD/ST/V/A):
    SP : [wait V>=(T-1)*8]  load pkt[T%2]            .inc LD 16
    DVE: [wait LD>=16(T+1)] [wait ST>=16(T-1)]
         per g: mul1, reduce_d, +mask, -max(.inc V)
                [wait A] recip, mul2, reduce_u, scale+item(.inc V)
    ACT: per g: [wait V] exp(bias=-max, accum=esum)  .inc A
         [wait V>=T*8+8] store outt[T%2]             .inc ST 16
"""

import sys

if "/opt/trn_rl_repo" not in sys.path:
    sys.path.insert(0, "/opt/trn_rl_repo")

import numpy as np

import concourse.bass as bass
from concourse import mybir
from concourse.bass_utils import run_bass_kernel_spmd


def _install_ntff_hook_shim():
    """The container's antenv lacks axon_hooks; recreate it so
    run_bass_kernel_spmd(trace=True) can capture NTFF profiles through
    libaxon_pjrt.so (same ctypes path trn_boot uses)."""
    import contextlib
    import ctypes
    import types

    if "antenv.axon_hooks" in sys.modules:
        return
    so_path = "/opt/axon/libaxon_pjrt.so"
    try:
        lib = ctypes.CDLL(so_path)
    except OSError:
        return
    if not hasattr(lib, "axon_start_nrt_profile"):
        return
    lib.axon_start_nrt_profile.argtypes = [
        ctypes.POINTER(ctypes.c_int64),
        ctypes.c_size_t,
    ]
    lib.axon_start_nrt_profile.restype = ctypes.c_int64
    lib.axon_stop_nrt_profile.argtypes = [ctypes.c_char_p]
    lib.axon_stop_nrt_profile.restype = ctypes.c_int64

    @contextlib.contextmanager
    def _hook(output_dir, device_ids):
        import jax

        jax.devices()
        if device_ids:
            ids = (ctypes.c_int64 * len(device_ids))(*device_ids)
            rc = lib.axon_start_nrt_profile(ids, len(device_ids))
        else:
            rc = lib.axon_start_nrt_profile(None, 0)
        if rc != 0:
            raise RuntimeError(f"axon_start_nrt_profile rc={rc}")
        try:
            yield
        finally:
            n = lib.axon_stop_nrt_profile(str(output_dir).encode())
            print(f"ntff profile: {n} file(s) written to {output_dir}")

    mod = types.ModuleType("antenv.axon_hooks")
    mod.get_axon_ntff_profile_hook = lambda: _hook
    mod.set_axon_ntff_profile_hook = lambda h: None
    sys.modules["antenv.axon_hooks"] = mod


_install_ntff_hook_shim()

# ---------------------------------------------------------------- config
B, S, U, D = 512, 200, 50, 32
NCORES = 8
BC = B // NCORES            # batches per core = 64
ROWS = BC * S               # rows per core = 12800
P = 128                     # SBUF partitions
G = 4                       # row-groups of 128 per DMA tile
NT = ROWS // (P * G)        # outer tiles per core = 25
NEG = -1e9

SIM_DT = "f32"              # "f32" | "bf16" (bf16 halves HBM traffic for sim+user)

UD = U * D                  # 1600
ROWW = UD + D + U + D       # packed row width (f32 words) = 1714


def _audit_waits(nc, max_waits=1):
    bad = []
    for blk in nc.m.functions[0].blocks:
        for ins in blk.instructions:
            si = ins.sync_info
            if si is not None and len(si.on_wait) > max_waits:
                bad.append((blk.name, ins.name, ins.opcode, len(si.on_wait)))
    if bad:
        msg = "\n".join(f"  {b}/{n} {o}: {k} waits" for b, n, o, k in bad)
        raise RuntimeError(f"instructions exceeding {max_waits} sync wait(s):\n{msg}")


# ---------------------------------------------------------------- kernel IR
def _build_nc():
    f32 = mybir.dt.float32
    nc = bass.Bass()

    pk_d = nc.dram_tensor("pk", [ROWS, ROWW], f32, kind="ExternalInput")
    out_d = nc.dram_tensor("out", [ROWS, D], f32, kind="ExternalOutput")

    pk_v = pk_d[:].rearrange("(T g p) f -> T p g f", g=G, p=P)
    out_v = out_d[:].rearrange("(T g p) f -> T p g f", g=G, p=P)

    o_user, o_maskf, o_item = UD, UD + D, UD + D + U

    # SBUF buffers
    pkt = [nc.alloc_sbuf_tensor(f"pkt{i}", [P, G * ROWW], f32) for i in range(3)]
    tmp = nc.alloc_sbuf_tensor("tmp", [P, U, D], f32)
    tmp2 = nc.alloc_sbuf_tensor("tmp2", [P, U, D], f32)
    scores = nc.alloc_sbuf_tensor("scores", [P, U], f32)
    scoresm = [nc.alloc_sbuf_tensor(f"scoresm{i}", [P, U], f32) for i in range(2)]
    e = [nc.alloc_sbuf_tensor(f"e{i}", [P, U], f32) for i in range(2)]
    esum = [nc.alloc_sbuf_tensor(f"esum{i}", [P, 1], f32) for i in range(2)]
    recip = nc.alloc_sbuf_tensor("recip", [P, 1], f32)
    outw = nc.alloc_sbuf_tensor("outw", [P, D], f32)
    outt = [nc.alloc_sbuf_tensor(f"outt{i}", [P, G * D], f32) for i in range(2)]

    LD = nc.alloc_semaphore("LD")
    ST = nc.alloc_semaphore("ST")
    V = nc.alloc_semaphore("V")
    A = nc.alloc_semaphore("A")

    # V tick values, per tile T (8 ticks, in DVE emission order):
    #   P1(g) ends with the mask-add tick, P2(g) ends with the stt tick.
    #   emission: P1(0) P1(1) P2(0) P1(2) P2(1) P1(3) P2(2) P2(3)
    _P1_TICK = {0: 1, 1: 2, 2: 4, 3: 6}
    _P2_TICK = {0: 3, 1: 5, 2: 7, 3: 8}

    with nc.Block() as blk:

        @blk.sync
        def _(sp):
            for T in range(NT):
                if T >= 3:
                    # pkt slot WAR: DVE finished reading tile T-3
                    sp.wait_ge(V, (T - 2) * 8)
                sp.dma_start(out=pkt[T % 3][:], in_=pk_v[T]).then_inc(LD, 16)

        def P1(v, T, g):
            pk2 = pkt[T % 3][:].rearrange("p (g w) -> p g w", g=G)
            sim3 = pk2[:, g, :UD].rearrange("p (u d) -> p u d", d=D)
            usert = pk2[:, g, o_user : o_user + D]
            maskt = pk2[:, g, o_maskf : o_maskf + U]
            ub = usert.unsqueeze(1).broadcast_to([P, U, D])
            v.tensor_mul(tmp[:], sim3, ub)
            v.tensor_reduce(
                scores[:], tmp[:],
                axis=mybir.AxisListType.X, op=mybir.AluOpType.add,
            )
            v.tensor_add(scoresm[g % 2][:], scores[:], maskt).then_inc(V, 1)

        def P2(v, T, g):
            pk2 = pkt[T % 3][:].rearrange("p (g w) -> p g w", g=G)
            sim3 = pk2[:, g, :UD].rearrange("p (u d) -> p u d", d=D)
            itemt = pk2[:, g, o_item : o_item + D]
            v.wait_ge(A, T * G + g + 1)
            v.reciprocal(recip[:], esum[g % 2][:])
            ebc = e[g % 2][:].unsqueeze(2).broadcast_to([P, U, D])
            v.tensor_mul(tmp2[:], sim3, ebc)
            v.tensor_reduce(
                outw[:], tmp2[:].rearrange("p u d -> p d u"),
                axis=mybir.AxisListType.X, op=mybir.AluOpType.add,
            )
            v.scalar_tensor_tensor(
                out=outt[T % 2][:, g * D : (g + 1) * D],
                in0=outw[:], scalar=recip[:], in1=itemt,
                op0=mybir.AluOpType.mult, op1=mybir.AluOpType.add,
            ).then_inc(V, 1)

        @blk.vector
        def _(v):
            for T in range(NT):
                v.wait_ge(LD, 16 * (T + 1))
                if T >= 2:
                    # outt slot WAR: store of tile T-2 completed
                    v.wait_ge(ST, 16 * (T - 1))
                # software pipeline: exp(g) overlaps P1(g+1)
                P1(v, T, 0)
                P1(v, T, 1)
                P2(v, T, 0)
                P1(v, T, 2)
                P2(v, T, 1)
                P1(v, T, 3)
                P2(v, T, 2)
                P2(v, T, 3)

        @blk.scalar
        def _(a):
            for T in range(NT):
                for g in range(G):
                    a.wait_ge(V, T * 8 + _P1_TICK[g])
                    # scores are O(30) max: exp is fp32-safe without the
                    # usual -max bias; masked entries underflow to 0.
                    a.activation(
                        e[g % 2][:], scoresm[g % 2][:],
                        mybir.ActivationFunctionType.Exp,
                        accum_out=esum[g % 2][:],
                    ).then_inc(A, 1)
                a.wait_ge(V, T * 8 + 8)
                a.dma_start(
                    out=out_v[T],
                    in_=outt[T % 2][:].rearrange("p (g w) -> p g w", g=G),
                ).then_inc(ST, 16)

    _audit_waits(nc)
    return nc


_NC_CACHE = {}


def _get_nc():
    key = (SIM_DT, G)
    if key not in _NC_CACHE:
        _NC_CACHE[key] = _build_nc()
    return _NC_CACHE[key]


# ---------------------------------------------------------------- host side
def _prep_core_inputs(current_user_embedding, similar_user_embedding,
                      current_item_embedding, mask):
    in_maps = []
    for c in range(NCORES):
        b0, b1 = c * BC, (c + 1) * BC
        pk = np.empty((ROWS, ROWW), dtype=np.float32)
        pk[:, :UD] = similar_user_embedding[b0:b1].reshape(ROWS, UD)
        pk[:, o_user_np : o_user_np + D] = np.broadcast_to(
            current_user_embedding[b0:b1, None, :], (BC, S, D)
        ).reshape(ROWS, D)
        pk[:, o_maskf_np : o_maskf_np + U] = np.where(
            mask[b0:b1], np.float32(NEG), np.float32(0.0)
        ).reshape(ROWS, U)
        pk[:, o_item_np:] = current_item_embedding[b0:b1].reshape(ROWS, D)
        in_maps.append({"pk": pk})
    return in_maps


o_user_np, o_maskf_np, o_item_np = UD, UD + D, UD + D + U


def _run(inputs, trace=False):
    nc = _get_nc()
    in_maps = _prep_core_inputs(**inputs)
    res = run_bass_kernel_spmd(
        nc, in_maps, core_ids=list(range(NCORES)), trace=trace
    )
    out = np.empty((B, S, D), dtype=np.float32)
    for c in range(NCORES):
        out[c * BC : (c + 1) * BC] = res.results[c]["out"].reshape(BC, S, D)
    return out, res


def kernel(**inputs):
    out, _ = _run(inputs, trace=False)
    return out



# revision 23
# speedup vs baseline: 2.7657x; 2.7657x over previous
"""Trainium2 Bass kernel for masked similar-user attention (v2: PE-scores).

Computation (per batch b, position s):
    scores[u] = dot(user[b], sim[b,s,u,:])        (u = 50, d = 32)
    scores    = where(mask, -1e9, scores)
    attn      = softmax(scores)
    out[s]    = sum_u attn[u] * sim[b,s,u,:] + item[b,s]

Sharding: pure data parallel over batch (B=512 -> 64 per core, 8 cores).

Design (per core):
  * sim is packed host-side as [128, 16, S*U] bf16 with partition
    p = delta*64 + b and tile t covering dim d = 2t+delta.  One HBM load,
    used twice on-chip.
  * scores via TensorE: 16 accumulating matmuls per s-chunk with
    block-diagonal user stationaries (lhsT[p, m] = user[b_m, 2t+delta]
    iff b_p == m), k = (delta,b) = 128 -> psum[64, chunk*U].  A 17th
    matmul with an identity stationary adds the -1e9 mask bias.
  * ScalarE: exp psum -> e (bf16) in SBUF.
  * DVE: esum (segmented reduce) + reciprocal; delta-copy of e to the
    upper 64 partitions; then per t: tensor_mul (bf16 2x mode) with sim
    and an aligned pairwise add-tree over u (50 = 18+14 | 16 | 8 | 4 | 2)
    -> weighted sums; fold 1/esum and item on the way out.
  * Output [128, 16, S] f32, reassembled host-side.

Pipeline: s-chunks of 10 positions (psum bank granularity), superchunks
of 4 chunks (DVE granularity), 2 sim buffers: DMA of superchunk k+2
overlaps DVE P2 of superchunk k+1 while PE/ACT run ahead.
"""

import sys

if "/opt/trn_rl_repo" not in sys.path:
    sys.path.insert(0, "/opt/trn_rl_repo")

import numpy as np

import concourse.bass as bass
from concourse import mybir
from concourse.bass_utils import run_bass_kernel_spmd


def _install_ntff_hook_shim():
    """The container's antenv lacks axon_hooks; recreate it so
    run_bass_kernel_spmd(trace=True) can capture NTFF profiles through
    libaxon_pjrt.so (same ctypes path trn_boot uses)."""
    import contextlib
    import ctypes
    import types

    if "antenv.axon_hooks" in sys.modules:
        return
    so_path = "/opt/axon/libaxon_pjrt.so"
    try:
        lib = ctypes.CDLL(so_path)
    except OSError:
        return
    if not hasattr(lib, "axon_start_nrt_profile"):
        return
    lib.axon_start_nrt_profile.argtypes = [
        ctypes.POINTER(ctypes.c_int64),
        ctypes.c_size_t,
    ]
    lib.axon_start_nrt_profile.restype = ctypes.c_int64
    lib.axon_stop_nrt_profile.argtypes = [ctypes.c_char_p]
    lib.axon_stop_nrt_profile.restype = ctypes.c_int64

    @contextlib.contextmanager
    def _hook(output_dir, device_ids):
        import jax

        jax.devices()
        if device_ids:
            ids = (ctypes.c_int64 * len(device_ids))(*device_ids)
            rc = lib.axon_start_nrt_profile(ids, len(device_ids))
        else:
            rc = lib.axon_start_nrt_profile(None, 0)
        if rc != 0:
            raise RuntimeError(f"axon_start_nrt_profile rc={rc}")
        try:
            yield
        finally:
            n = lib.axon_stop_nrt_profile(str(output_dir).encode())
            print(f"ntff profile: {n} file(s) written to {output_dir}")

    mod = types.ModuleType("antenv.axon_hooks")
    mod.get_axon_ntff_profile_hook = lambda: _hook
    mod.set_axon_ntff_profile_hook = lambda h: None
    sys.modules["antenv.axon_hooks"] = mod


_install_ntff_hook_shim()

# ---------------------------------------------------------------- config
B, S, U, D = 512, 200, 50, 32
NCORES = 8
BC = B // NCORES            # batches per core = 64
T16 = D // 2                # d-pair tiles = 16
SU = S * U                  # 10000
CH = 10                     # s positions per chunk (psum bank granularity)
CHF = CH * U                # 500 free columns per chunk
NCH = S // CH               # 20 chunks
SCH = 4                     # chunks per superchunk
NSC = NCH // SCH            # 5 superchunks
SCS = SCH * CH              # 40 s per superchunk
SCF = SCH * CHF             # 2000 free columns per superchunk
NPS = 4                     # psum banks in flight
NEG = -1e9

BF16 = mybir.dt.bfloat16
F32 = mybir.dt.float32
NP_BF16 = mybir.dt.np(BF16)


def _audit_waits(nc, max_waits=1):
    bad = []
    for blk in nc.m.functions[0].blocks:
        for ins in blk.instructions:
            si = ins.sync_info
            if si is not None and len(si.on_wait) > max_waits:
                bad.append((blk.name, ins.name, ins.opcode, len(si.on_wait)))
    if bad:
        msg = "\n".join(f"  {b}/{n} {o}: {k} waits" for b, n, o, k in bad)
        raise RuntimeError(f"instructions exceeding {max_waits} sync wait(s):\n{msg}")


DEBUG_DUMPS = False


# ---------------------------------------------------------------- kernel IR
def _build_nc():
    nc = bass.Bass()

    simt_d = nc.dram_tensor("simt", [128, T16, SU], BF16, kind="ExternalInput")
    maskb_d = nc.dram_tensor("maskb", [BC, SU], BF16, kind="ExternalInput")
    itemt_d = nc.dram_tensor("itemt", [128, T16, S], F32, kind="ExternalInput")
    userd_d = nc.dram_tensor("userd", [128, T16, 128], BF16, kind="ExternalInput")
    ident_d = nc.dram_tensor("ident", [BC, 128], BF16, kind="ExternalInput")
    out_d = nc.dram_tensor("out", [128, T16, S], F32, kind="ExternalOutput")

    # SBUF
    simb = [nc.alloc_sbuf_tensor(f"simb{i}", [128, SCH, T16, CHF], BF16)
            for i in range(2)]
    maskc = [nc.alloc_sbuf_tensor(f"maskc{i}", [BC, SCH, CHF], BF16)
             for i in range(2)]
    erep = [nc.alloc_sbuf_tensor(f"erep{i}", [128, SCF], BF16) for i in range(2)]
    userb = nc.alloc_sbuf_tensor("userb", [128, T16, 128], BF16)
    identb = nc.alloc_sbuf_tensor("identb", [BC, 128], BF16)
    itemb = nc.alloc_sbuf_tensor("itemb", [128, T16, S], F32)
    outb = nc.alloc_sbuf_tensor("outb", [128, T16, S], F32)
    esum = nc.alloc_sbuf_tensor("esum", [128, S], F32)
    rq = nc.alloc_sbuf_tensor("rq", [128, S], F32)
    rrep = None  # scores are PE-duplicated to all 128 partitions instead
    dbgA = {}
    if DEBUG_DUMPS:
        dbgA["dbg_rq0"] = nc.alloc_sbuf_tensor("dbg_rq0", [128, SCS], F32)
        dbgA["dbg_ow_k0"] = nc.alloc_sbuf_tensor("dbg_ow_k0", [128, SCS], F32)
        dbgA["dbg_ow2_k0"] = nc.alloc_sbuf_tensor("dbg_ow2_k0", [128, SCS], F32)
        dbgA["dbg_sim_k0"] = nc.alloc_sbuf_tensor("dbg_sim_k0", [128, CHF], BF16)
        dbgA["dbg_erep_k0"] = nc.alloc_sbuf_tensor("dbg_erep_k0", [128, SCF], BF16)
    tmp2 = nc.alloc_sbuf_tensor("tmp2", [128, SCF], BF16)
    tA = nc.alloc_sbuf_tensor("tA", [128, SCS, 32], BF16)
    tB = nc.alloc_sbuf_tensor("tB", [128, SCS, 16], BF16)
    ow = nc.alloc_sbuf_tensor("ow", [128, SCS], F32)
    ow2 = nc.alloc_sbuf_tensor("ow2", [128, SCS], F32)

    # PSUM
    psum = [nc.alloc_psum_tensor(f"ps{i}", [128, CHF], F32) for i in range(NPS)]

    LD = nc.alloc_semaphore("LD")   # DMA-in completions
    PS = nc.alloc_semaphore("PS")   # PE chunk done
    EX = nc.alloc_semaphore("EX")   # exp chunk done
    P2 = nc.alloc_semaphore("P2")   # DVE superchunk done
    ST = nc.alloc_semaphore("ST")   # out DMA completions

    with nc.Block() as blk:

        @blk.sync
        def _(sp):
            sp.dma_start(out=userb[:], in_=userd_d[:]).then_inc(LD, 16)
            sp.dma_start(out=identb[:], in_=ident_d[:]).then_inc(LD, 16)
            sp.dma_start(out=itemb[:], in_=itemt_d[:]).then_inc(LD, 16)
            for i in range(NCH):
                k, c = divmod(i, SCH)
                sb = k % 2
                if k >= 2 and c == 0:
                    sp.wait_ge(P2, k - 1)
                sp.dma_start(
                    out=simb[sb][:][:, c, :, :],
                    in_=simt_d[:][:, :, i * CHF : (i + 1) * CHF],
                ).then_inc(LD, 16)
                sp.dma_start(
                    out=maskc[sb][:][:, c, :],
                    in_=maskb_d[:][:, i * CHF : (i + 1) * CHF],
                ).then_inc(LD, 16)

        @blk.tensor
        def _(pe):
            for i in range(NCH):
                k, c = divmod(i, SCH)
                sb = k % 2
                pb = i % NPS
                pe.wait_ge(LD, 16 * (3 + 2 * (i + 1)))
                if i >= NPS:
                    pe.wait_ge(EX, i - NPS + 1)
                for t in range(T16):
                    pe.matmul(
                        psum[pb][:],
                        lhsT=userb[:][:, t, :],
                        rhs=simb[sb][:][:, c, t, :],
                        start=(t == 0),
                        stop=False,
                    )
                pe.matmul(
                    psum[pb][:],
                    lhsT=identb[:],
                    rhs=maskc[sb][:][:, c, :],
                    start=False,
                    stop=True,
                ).then_inc(PS, 1)

        @blk.scalar
        def _(a):
            for i in range(NCH):
                k, c = divmod(i, SCH)
                sb = k % 2
                if k >= 2 and c == 0:
                    a.wait_ge(P2, k - 1)
                a.wait_ge(PS, i + 1)
                a.activation(
                    erep[sb][:][:, c * CHF : (c + 1) * CHF],
                    psum[i % NPS][:],
                    mybir.ActivationFunctionType.Exp,
                ).then_inc(EX, 1)
                if c == SCH - 1 and k >= 1:
                    # out DMA for the previous superchunk
                    a.wait_ge(P2, k)
                    a.dma_start(
                        out=out_d[:][:, :, (k - 1) * SCS : k * SCS],
                        in_=outb[:][:, :, (k - 1) * SCS : k * SCS],
                    ).then_inc(ST, 16)
            a.wait_ge(P2, NSC)
            a.dma_start(
                out=out_d[:][:, :, (NSC - 1) * SCS : NSC * SCS],
                in_=outb[:][:, :, (NSC - 1) * SCS : NSC * SCS],
            ).then_inc(ST, 16)
            if DEBUG_DUMPS:
                dbg = {
                    "dbg_esum": esum, "dbg_rq": rq, "dbg_erep0": erep[0],
                    "dbg_tmp2": tmp2, "dbg_ow": ow, "dbg_ow2": ow2,
                }
                dbg.update(dbgA)
                for nm, tens in dbg.items():
                    d = nc.dram_tensor(
                        nm + "_o", list(tens.shape), tens.dtype,
                        kind="ExternalOutput",
                    )
                    a.dma_start(out=d[:], in_=tens[:]).then_inc(ST, 16)

        @blk.vector
        def _(v):
            for k in range(NSC):
                sb = k % 2
                for c in range(SCH):
                    i = k * SCH + c
                    v.wait_ge(EX, i + 1)
                    ev = erep[sb][:][:, c * CHF : (c + 1) * CHF]
                    v.tensor_reduce(
                        esum[:][:, i * CH : (i + 1) * CH],
                        ev.rearrange("p (s u) -> p s u", u=U),
                        axis=mybir.AxisListType.X, op=mybir.AluOpType.add,
                    )
                # reciprocals batched after all 4 reduces: a reciprocal
                # issued back-to-back with its producing tensor_reduce reads
                # the reduce's tail elements before they land in SBUF.
                for c in range(SCH):
                    i = k * SCH + c
                    v.reciprocal(
                        rq[:][:, i * CH : (i + 1) * CH],
                        esum[:][:, i * CH : (i + 1) * CH],
                    )
                ks = slice(k * SCS, (k + 1) * SCS)
                ev3 = erep[sb][:].rearrange("p (c f) -> p c f", f=CHF)
                v3 = tmp2[:].rearrange("p (s u) -> p s u", u=U)
                for t in range(T16):
                    v.tensor_mul(
                        tmp2[:].rearrange("p (c f) -> p c f", f=CHF),
                        simb[sb][:][:, :, t, :],
                        ev3,
                    )
                    # aligned pairwise tree over u: 50 -> 32 -> 16 -> 8 -> 4 -> 2 -> 1
                    v.tensor_add(tA[:][:, :, 0:18], v3[:, :, 0:18], v3[:, :, 32:50])
                    v.tensor_copy(tA[:][:, :, 18:32], v3[:, :, 18:32])
                    v.tensor_add(tB[:], tA[:][:, :, 0:16], tA[:][:, :, 16:32])
                    v.tensor_add(tA[:][:, :, 0:8], tB[:][:, :, 0:8], tB[:][:, :, 8:16])
                    v.tensor_add(tB[:][:, :, 0:4], tA[:][:, :, 0:4], tA[:][:, :, 4:8])
                    v.tensor_add(tA[:][:, :, 0:2], tB[:][:, :, 0:2], tB[:][:, :, 2:4])
                    v.tensor_add(
                        ow[:].unsqueeze(2), tA[:][:, :, 0:1], tA[:][:, :, 1:2]
                    )
                    v.tensor_mul(ow2[:], ow[:], rq[:][:, ks])
                    ins = v.tensor_add(
                        outb[:][:, t, ks], ow2[:], itemb[:][:, t, ks]
                    )
                    if t == T16 - 1:
                        if DEBUG_DUMPS and k == 0:
                            v.tensor_copy(dbgA["dbg_rq0"][:], rq[:][:, 0:SCS])
                            v.tensor_copy(dbgA["dbg_ow_k0"][:], ow[:])
                            v.tensor_copy(dbgA["dbg_ow2_k0"][:], ow2[:])
                            v.tensor_copy(dbgA["dbg_sim_k0"][:],
                                          simb[0][:][:, 0, T16 - 1, :])
                            v.tensor_copy(dbgA["dbg_erep_k0"][:], erep[0][:])
                        ins.then_inc(P2, 1)

    _audit_waits(nc)
    return nc


_NC_CACHE = {}


def _get_nc():
    if "nc" not in _NC_CACHE:
        _NC_CACHE["nc"] = _build_nc()
    return _NC_CACHE["nc"]


# ---------------------------------------------------------------- host side
def _prep_core_inputs(current_user_embedding, similar_user_embedding,
                      current_item_embedding, mask):
    ident = np.zeros((BC, 128), dtype=NP_BF16)
    ident[np.arange(BC), np.arange(BC)] = 1
    ident[np.arange(BC), BC + np.arange(BC)] = 1
    in_maps = []
    for cidx in range(NCORES):
        b0, b1 = cidx * BC, (cidx + 1) * BC
        sim_c = np.asarray(similar_user_embedding[b0:b1], dtype=np.float32)
        user_c = np.asarray(current_user_embedding[b0:b1], dtype=np.float32)
        item_c = np.asarray(current_item_embedding[b0:b1], dtype=np.float32)
        mask_c = np.asarray(mask[b0:b1])

        # simt[p, t, s*U+u] = sim[b, s, u, 2t+delta], p = delta*64+b
        x = sim_c.transpose(3, 0, 1, 2)            # [d, b, s, u]
        x = x.reshape(T16, 2, BC, SU)              # [t, delta, b, su]
        x = x.transpose(1, 2, 0, 3)                # [delta, b, t, su]
        simt = np.ascontiguousarray(x.reshape(128, T16, SU)).astype(NP_BF16)

        # userd[p, t, m] = user[b_m%64, 2t+delta_p] iff (p % 64) == (m % 64)
        # (columns duplicated so scores land on all 128 psum partitions)
        uc = user_c.T.reshape(T16, 2, BC)          # [t, delta, b]
        uc = uc.transpose(1, 2, 0).reshape(128, T16)   # [p, t]
        userd = np.zeros((128, T16, 128), dtype=NP_BF16)
        pidx = np.arange(128)
        userd[pidx, :, pidx % BC] = uc.astype(NP_BF16)
        userd[pidx, :, BC + pidx % BC] = uc.astype(NP_BF16)

        maskb = np.where(mask_c, np.float32(NEG), np.float32(0.0))
        maskb = maskb.reshape(BC, SU).astype(NP_BF16)

        y = item_c.transpose(2, 0, 1)              # [d, b, s]
        y = y.reshape(T16, 2, BC, S).transpose(1, 2, 0, 3)  # [delta, b, t, s]
        itemt = np.ascontiguousarray(y.reshape(128, T16, S), dtype=np.float32)

        in_maps.append({
            "simt": simt, "maskb": maskb, "itemt": itemt,
            "userd": userd, "ident": ident,
        })
    return in_maps


def _unpack_out(r):
    # r: [128, T16, S] f32 -> out[b, s, 2t+delta]
    return r.reshape(2, BC, T16, S).transpose(1, 3, 2, 0).reshape(BC, S, D)


def _run(inputs, trace=False):
    nc = _get_nc()
    in_maps = _prep_core_inputs(**inputs)
    res = run_bass_kernel_spmd(
        nc, in_maps, core_ids=list(range(NCORES)), trace=trace
    )
    out = np.empty((B, S, D), dtype=np.float32)
    for cidx in range(NCORES):
        out[cidx * BC : (cidx + 1) * BC] = _unpack_out(
            np.asarray(res.results[cidx]["out"], dtype=np.float32)
        )
    return out, res


def kernel(**inputs):
    out, _ = _run(inputs, trace=False)
    return out
